# revision 9
# baseline (speedup 1.0000x reference)
"""Trainium2 Bass kernel for nn_MultiHeadAttention_8074538516581.

Sharding: 8 cores = batch(4) x head-group(2 groups of 6 heads).
Each core: qkv projection for its 6 heads (bf16 matmuls, fp32 psum; the
struct-embed term is pre-added into x on the host in fp32), per-head
attention with the reference's exact semantics (q/k rounded to bf16,
shift-free softmax -- the row-max subtraction cancels in the
normalization; the [-30,30] logit clip and the 1e5/1e-10 guards are
provably inactive here), and an E-major partial output projection over
its 384 head-dims.  Host sums the two head-group partials per batch,
transposes, and adds b_out.

Token permutation: queries with (t % 64) % 3 == 0 are zeroed by the
reference's load mask, making their attention output mean(v) per head.
Tokens are permuted live-first so the 672 live queries are contiguous;
one pinned-zero query column (output exactly mean(v)) is broadcast to
the 351 masked tokens on the host.

Schedule: the ACT engine's 48 exp instructions (~0.7us each) and the PE
engine's matmul stream are the two near-critical resources.  The kt
loop keeps ACT saturated (scores one slot ahead through a 2-deep psum
rotation) while PE "fill" work is interleaved into the exp-gated
slack: during c0's attention the V projection and the c1 k/q slices,
during c1's the c2 k/q slices, during c2's the c0+c1 output-projection
partials.  PSUM budget: scores 2x2 banks + 2 accumulator banks + one
rotating 2-bank aux window for everything else.  The tail finishes the
output projection per E-chunk through a 3-deep psum pipeline, shipping
each [128, 673] slab as soon as it is ready."""
import numpy as np
import ml_dtypes
from contextlib import ExitStack
from itertools import chain

import concourse.bass as bass
import concourse.mybir as mybir
import concourse.tile as tile
from concourse import bacc
from concourse.bass import ts
from concourse.bass_utils import run_bass_kernel_spmd

B, T, E = 4, 1024, 768
H, D = 12, 64
HG = 6                  # heads per group
GD = HG * D             # 384 head-dims per group
BLOCK_M = 64
LIVE = 672              # tokens with (t % BLOCK_M) % 3 != 0
NQ = LIVE + 1           # live queries + one pinned-zero query (= mean-v row)
SCALE = 1.0 / 8.0       # 1/sqrt(64)
QC = 6                  # q chunks of 128 (last holds 33 live+pinned cols)
NPP = 5                 # rotating exp-output buffers

BF16 = mybir.dt.bfloat16
F32 = mybir.dt.float32

_perm = None
_nc = None

TRACE = False
LAST_RES = None


def _perm_live_first():
    t = np.arange(T)
    m = (t % BLOCK_M) % 3 == 0
    return np.concatenate([t[~m], t[m]])


def _build_bass(debug=False):
    nc = bacc.Bacc()
    xT_d = nc.dram_tensor("xT", [E, T], BF16, kind="ExternalInput")
    wq_d = nc.dram_tensor("wq", [E, GD], BF16, kind="ExternalInput")
    wk_d = nc.dram_tensor("wk", [E, GD], BF16, kind="ExternalInput")
    wv_d = nc.dram_tensor("wv", [E, GD], BF16, kind="ExternalInput")
    woT_d = nc.dram_tensor("woT", [GD, E], BF16, kind="ExternalInput")
    outT_d = nc.dram_tensor("outT", [E, NQ], BF16, kind="ExternalOutput")
    if debug:
        dbg = {nm: nc.dram_tensor(nm, sh, dt, kind="ExternalOutput")
               for nm, sh, dt in (
                   ("dbg_q", [128, 3, NQ], BF16),
                   ("dbg_k", [128, 3, T], BF16),
                   ("dbg_v", [128, 8, HG * (D + 1)], BF16),
                   ("dbg_attnT", [128, 3, QC * 128], BF16),
                   ("dbg_out01", [128, 3, NQ], BF16),
               )}

    xT_r = xT_d[:, :].rearrange("(c p) t -> p c t", p=128)
    wq_r = wq_d[:, :].rearrange("(c p) n -> p c n", p=128)
    wk_r = wk_d[:, :].rearrange("(c p) n -> p c n", p=128)
    wv_r = wv_d[:, :].rearrange("(c p) n -> p c n", p=128)
    woT_r = woT_d[:, :].rearrange("(c p) n -> p c n", p=128)

    Exp = mybir.ActivationFunctionType.Exp

    with tile.TileContext(nc) as tc:
        with tc.tile_pool(name="singles", bufs=1) as S:
            xT_sb = S.tile([128, 6, T], BF16)
            wq_sb = S.tile([128, 6, GD], BF16)
            wk_sb = S.tile([128, 6, GD], BF16)
            wv_sb = S.tile([128, 6, GD], BF16)
            woT_sb = S.tile([128, 3, E], BF16)
            warm_sb = S.tile([128, 512], BF16)
            qT_sb = S.tile([128, 3, NQ], BF16)
            kT_sb = S.tile([128, 3, T], BF16)
            v_sb = S.tile([128, 8, HG * (D + 1)], BF16)   # per-head [v | 1]
            attnT_sb = S.tile([128, 3, QC * 128], BF16)
            out01_sb = S.tile([128, 3, NQ], BF16)         # c0+c1 out partials
            dpre_sb = S.tile([1, 1], F32)
            ppb = [S.tile([128, 768], BF16, tag=f"ppb{j}", name=f"ppb{j}")
                   for j in range(NPP)]

            # ---- input DMAs (DMA engines are a serial resource: order by
            # first consumer: x chunks + wk/wq gate the first exp; wv gates
            # the V fills; woT only the out-projection)
            nc.sync.dma_start(out=xT_sb[:, 0:2, :], in_=xT_r[:, 0:2, :])
            nc.sync.dma_start(out=wk_sb, in_=wk_r)
            nc.sync.dma_start(out=xT_sb[:, 2:4, :], in_=xT_r[:, 2:4, :])
            nc.sync.dma_start(out=wq_sb, in_=wq_r)
            nc.sync.dma_start(out=xT_sb[:, 4:6, :], in_=xT_r[:, 4:6, :])
            nc.sync.dma_start(out=wv_sb, in_=wv_r)
            nc.sync.dma_start(out=woT_sb, in_=woT_r)

            nc.vector.memset(warm_sb, 0.5)
            for j in range(NPP):
                # pad cols so pv's last q-chunk stationary reads defined
                # small values (keeps garbage rows finite)
                nc.vector.memset(ppb[j][:, NQ:768], 1e-10)
            v_ones = v_sb[:, :, :].rearrange(
                "p a (h e) -> p a h e", e=D + 1)[:, :, :, D:D + 1]
            nc.vector.memset(v_ones, 1.0)
            # pinned-zero query column (mean-v row for masked tokens)
            nc.vector.memset(qT_sb[:, :, LIVE:NQ], 0.0)
            # preload the exp table while DMAs run
            nc.scalar.activation(dpre_sb, warm_sb[0:1, 0:1], Exp)

            # ---------- helpers ----------
            def vcopy(tt, t):
                dst = v_sb[:, tt, :].rearrange(
                    "p (h e) -> p h e", e=D + 1)[:, :, 0:D]
                src = t[:, :].rearrange("p (h d) -> p h d", d=D)
                nc.vector.tensor_copy(dst, src)

            def kq_half(pool, c, which, half, eng="vector"):
                """One half-slice of the k or q projection for chunk c.
                k: tokens [0:512] / [512:1024]; q: [0:512] / [512:672].
                6 matmuls accumulating over E-chunks + one copy."""
                n = T if which == "k" else LIVE
                s0, s1 = (0, 512) if half == 0 else (512, n)
                w_sb = wk_sb if which == "k" else wq_sb
                dstT = kT_sb if which == "k" else qT_sb
                p = pool.tile([128, 512], F32, tag="aux",
                              name=f"kq_{which}{c}_{half}")
                for ek in range(6):
                    nc.tensor.matmul(p[:, 0:s1 - s0],
                                     w_sb[:, ek, ts(c, 128)],
                                     xT_sb[:, ek, s0:s1],
                                     start=(ek == 0), stop=(ek == 5))
                    if ek == 2:
                        yield
                if eng == "vector":
                    nc.vector.tensor_copy(dstT[:, c, s0:s1], p[:, 0:s1 - s0])
                else:
                    nc.scalar.copy(dstT[:, c, s0:s1], p[:, 0:s1 - s0])
                yield

            def v_tt(pool, tt):
                t = pool.tile([128, GD], F32, tag="aux", name=f"vt{tt}")
                for ek in range(6):
                    nc.tensor.matmul(t, xT_sb[:, ek, ts(tt, 128)],
                                     wv_sb[:, ek, :],
                                     start=(ek == 0), stop=(ek == 5))
                    if ek == 2:
                        yield
                vcopy(tt, t)
                yield

            def op_prerun(pool, ec):
                """c0+c1 contribution of out-projection E-chunk ec, staged
                to SBUF bf16 (c2 is added in the tail)."""
                p = pool.tile([128, NQ], F32, tag="aux", name=f"opp{ec}")
                for c3 in (0, 1):
                    for s0, s1 in ((0, 512), (512, NQ)):
                        nc.tensor.matmul(p[:, s0:s1],
                                         woT_sb[:, c3, ts(ec, 128)],
                                         attnT_sb[:, c3, s0:s1],
                                         start=(c3 == 0),
                                         stop=(c3 == 1),
                                         skip_group_check=True)
                    yield
                nc.vector.tensor_copy(out01_sb[:, ec, :], p)
                yield

            # ---------- pre-attention: warm + k/q chunk 0 ----------
            with ExitStack() as pre:
                kqp = pre.enter_context(
                    tc.tile_pool(name="kqp", bufs=2, space="PSUM"))
                warm_t = kqp.tile([128, 512], F32, tag="aux", name="warm_t")
                for _ in range(5):
                    nc.tensor.matmul(warm_t[:, 0:384], warm_sb[:, 0:128],
                                     warm_sb[:, 0:384], start=True, stop=True)
                # k chunk0 first half -> DVE copy; q halves -> ACT copies
                # (ACT is idle pre-attention; DVE takes the rest)
                for _ in kq_half(kqp, 0, "k", 0, "vector"):
                    pass
                for _ in kq_half(kqp, 0, "q", 0, "scalar"):
                    pass
                for _ in kq_half(kqp, 0, "q", 1, "scalar"):
                    pass

            with ExitStack() as stack:
                ps_s = stack.enter_context(
                    tc.tile_pool(name="ps_s", bufs=2, space="PSUM"))
                ps_acc = stack.enter_context(
                    tc.tile_pool(name="ps_acc", bufs=1, space="PSUM"))
                att_pool = stack.enter_context(
                    tc.tile_pool(name="att", bufs=2))
                rq_pool = stack.enter_context(
                    tc.tile_pool(name="rq", bufs=2))
                ppi = [0]

                def attention_c(c, fill, fill_quota):
                    accs = [ps_acc.tile([128, QC * (D + 1)], F32,
                                        tag=f"acc{i}", name=f"acc{c}_{i}")
                            for i in range(2)]

                    def pv(kt, i, pp):
                        h = 2 * c + i
                        vh = v_sb[:, kt, h * (D + 1):(h + 1) * (D + 1)]
                        for qc in range(QC):
                            nc.tensor.matmul(
                                accs[i][:, qc * (D + 1):(qc + 1) * (D + 1)],
                                pp[:, qc * 128:(qc + 1) * 128],
                                vh,
                                # one start=True per psum bank: it clears the
                                # whole bank's has_written so later regions'
                                # first writes overwrite rather than add
                                start=(kt == 0 and qc == 0), stop=(kt == 7),
                                skip_group_check=True)

                    pend = []
                    for kt in range(8):
                        for i in range(2):      # head 2c+i
                            po = i * 64
                            kh = kT_sb[po:po + 64, c, ts(kt, 128)]
                            qh = qT_sb[po:po + 64, c, :]
                            sp = ps_s.tile([128, T], F32, tag="s",
                                           name=f"s{c}_{kt}_{i}")
                            nc.tensor.matmul(sp[:, 0:512], kh, qh[:, 0:512],
                                             start=True, stop=True)
                            nc.tensor.matmul(sp[:, 512:NQ], kh, qh[:, 512:NQ],
                                             start=True, stop=True)
                            pp = ppb[ppi[0] % NPP]
                            ppi[0] += 1
                            nc.scalar.activation(pp[:, 0:NQ], sp[:, 0:NQ],
                                                 Exp, scale=SCALE)
                            pend.append((kt, i, pp))
                            while len(pend) > 3:
                                pv(*pend.pop(0))
                            for _ in range(fill_quota):
                                try:
                                    next(fill)
                                except StopIteration:
                                    break
                    # ---- normalize: per-partition recip + strided multiply
                    att = att_pool.tile([128, QC * 128], BF16, tag="att")

                    def norm(i):
                        rq = rq_pool.tile([128, QC], F32, tag=f"rq{i}")
                        a = accs[i]
                        den = bass.AP(tensor=a.tensor, offset=a.offset + D,
                                      ap=[list(a.ap[0])] + [[D + 1, QC]])
                        nc.vector.reciprocal(rq, den)
                        src = bass.AP(tensor=a.tensor, offset=a.offset,
                                      ap=[list(a.ap[0])] + [[D + 1, QC],
                                                            [1, D]])
                        sca = bass.AP(tensor=rq.tensor, offset=rq.offset,
                                      ap=[list(rq.ap[0])] + [[1, QC],
                                                             [0, D]])
                        dst = bass.AP(tensor=att.tensor,
                                      offset=att.offset + i * 64,
                                      ap=[list(att.ap[0])] + [[128, QC],
                                                              [1, D]])
                        nc.vector.tensor_mul(dst, src, sca)

                    while pend:
                        kt_, i_, pp_ = pend.pop(0)
                        pv(kt_, i_, pp_)
                        if not any(e[1] == i_ for e in pend):
                            norm(i_)
                    # ---- transpose -> attnT [dims, tok] on the XBAR DMA
                    # path (idle mid-kernel); c2 in two halves to cut the
                    # tail latency of its last chunk
                    o = attnT_sb[:, c, :]
                    if c < 2:
                        o3 = bass.AP(tensor=o.tensor, offset=o.offset,
                                     ap=[list(o.ap[0])] + [[128, QC],
                                                           [1, 128]])
                        nc.sync.dma_start_transpose(o3, att[:, :])
                    else:
                        for h0 in (0, 3):
                            oh = bass.AP(tensor=o.tensor,
                                         offset=o.offset + h0 * 128,
                                         ap=[list(o.ap[0])] + [[128, 3],
                                                               [1, 128]])
                            nc.sync.dma_start_transpose(
                                oh, att[:, h0 * 128:(h0 + 3) * 128])

                # c0's fills: k chunk0 2nd half, V for all 8 kt chunks,
                # then the c1 k/q slices (all 1-bank tiles rotating through
                # the 2-slot aux window).
                with ExitStack() as auxst:
                    vtp = auxst.enter_context(
                        tc.tile_pool(name="vtp", bufs=2, space="PSUM"))
                    f0 = chain(kq_half(vtp, 0, "k", 1),
                               *[v_tt(vtp, tt) for tt in range(8)],
                               kq_half(vtp, 1, "k", 0),
                               kq_half(vtp, 1, "k", 1),
                               kq_half(vtp, 1, "q", 0),
                               kq_half(vtp, 1, "q", 1))
                    attention_c(0, f0, 2)
                    for _ in f0:
                        pass
                with ExitStack() as auxst:
                    kq2 = auxst.enter_context(
                        tc.tile_pool(name="kq2", bufs=2, space="PSUM"))
                    f1 = chain(kq_half(kq2, 2, "k", 0),
                               kq_half(kq2, 2, "k", 1),
                               kq_half(kq2, 2, "q", 0),
                               kq_half(kq2, 2, "q", 1))
                    attention_c(1, f1, 2)
                    for _ in f1:
                        pass
                with ExitStack() as auxst:
                    opp = auxst.enter_context(
                        tc.tile_pool(name="opp", bufs=1, space="PSUM"))
                    f2 = chain(*[op_prerun(opp, ec) for ec in range(3)])
                    attention_c(2, f2, 2)
                    for _ in f2:
                        pass

            if debug:
                for nm, t in (("dbg_q", qT_sb), ("dbg_k", kT_sb),
                              ("dbg_v", v_sb), ("dbg_attnT", attnT_sb),
                              ("dbg_out01", out01_sb)):
                    nc.sync.dma_start(out=dbg[nm][:, :, :], in_=t[:, :, :])

            # ---------- tail: finish out-projection per E-chunk ----------
            with tc.tile_pool(name="opf", bufs=3, space="PSUM") as opf, \
                 tc.tile_pool(name="obp", bufs=3) as obp:
                for ec in range(6):
                    p = opf.tile([128, NQ], F32, tag="o", name=f"of{ec}")
                    cs = (2,) if ec < 3 else (0, 1, 2)
                    for c3 in cs:
                        for s0, s1 in ((0, 512), (512, NQ)):
                            nc.tensor.matmul(p[:, s0:s1],
                                             woT_sb[:, c3, ts(ec, 128)],
                                             attnT_sb[:, c3, s0:s1],
                                             start=(c3 == cs[0]),
                                             stop=(c3 == 2),
                                             skip_group_check=True)
                    ob = obp.tile([128, NQ], BF16, tag="ob", name=f"ob{ec}")
                    if ec < 3:
                        # add the preruned c0+c1 partial
                        nc.vector.tensor_add(ob, p, out01_sb[:, ec, :])
                    elif ec % 2 == 0:
                        nc.vector.tensor_copy(ob, p)
                    else:
                        nc.scalar.copy(ob, p)
                    nc.sync.dma_start(out=outT_d[ts(ec, 128), :], in_=ob)

    nc.finalize()
    return nc


def _get_bass():
    global _nc
    if _nc is None:
        _nc = _build_bass()
    return _nc


def kernel(x, idx, struct_embed, w_qkv, w_out, b_out):
    global _perm
    if _perm is None:
        _perm = _perm_live_first()
    perm = _perm

    x = np.asarray(x, dtype=np.float32)
    idx = np.asarray(idx)
    struct_embed = np.asarray(struct_embed, dtype=np.float32)
    w_qkv = np.asarray(w_qkv, dtype=np.float32)
    w_out = np.asarray(w_out, dtype=np.float32)
    b_out = np.asarray(b_out, dtype=np.float32)

    sid = ((idx == 1) * 1 + (idx == 2) * 2 + (idx == 3) * 3)  # [B,T]
    xs = x + struct_embed[sid]                                # fp32 pre-add

    bf = ml_dtypes.bfloat16
    in_maps = []
    for core in range(8):
        b, g = core // 2, core % 2
        sl = slice(g * GD, (g + 1) * GD)
        in_maps.append({
            "xT": np.ascontiguousarray(xs[b].T[:, perm]).astype(bf),
            "wq": np.ascontiguousarray(w_qkv[0 * E:1 * E][sl].T).astype(bf),
            "wk": np.ascontiguousarray(w_qkv[1 * E:2 * E][sl].T).astype(bf),
            "wv": np.ascontiguousarray(w_qkv[2 * E:3 * E][sl].T).astype(bf),
            "woT": np.ascontiguousarray(w_out[:, sl].T).astype(bf),
        })

    res = run_bass_kernel_spmd(_get_bass(), in_maps, core_ids=list(range(8)),
                               trace=TRACE)
    if TRACE:
        global LAST_RES
        LAST_RES = res

    inv = np.empty(T, dtype=np.int64)
    inv[perm] = np.arange(T)
    out = np.empty((B, T, E), dtype=np.float32)
    full = np.empty((T, E), dtype=np.float32)
    for b in range(B):
        acc = (res.results[2 * b]["outT"].astype(np.float32)
               + res.results[2 * b + 1]["outT"].astype(np.float32))
        # device computes permuted cols 0:NQ of outT; all masked-token
        # rows equal col LIVE (the pinned-zero query = mean-v output)
        full[0:NQ] = acc.T
        full[NQ:] = acc[:, LIVE]
        out[b] = full[inv] + b_out[None, :]
    return out


# revision 11
# speedup vs baseline: 1.0301x; 1.0301x over previous
"""Trainium2 Bass kernel for nn_MultiHeadAttention_8074538516581.

Sharding: 8 cores = batch(4) x head-group(2 groups of 6 heads).
Each core: qkv projection for its 6 heads (bf16 matmuls, fp32 psum; the
struct-embed term is pre-added into x on the host in fp32), per-head
attention with the reference's exact semantics (q/k rounded to bf16,
shift-free softmax -- the row-max subtraction cancels in the
normalization; the [-30,30] logit clip and the 1e5/1e-10 guards are
provably inactive here), and an E-major partial output projection over
its 384 head-dims.  Host sums the two head-group partials per batch,
transposes, and adds b_out.

Token permutation: queries with (t % 64) % 3 == 0 are zeroed by the
reference's load mask, making their attention output mean(v) per head.
Tokens are permuted live-first so the 672 live queries are contiguous;
one pinned-zero query column (output exactly mean(v)) is broadcast to
the 351 masked tokens on the host.

Schedule: the ACT engine's 48 exp instructions (~0.75us each) and the
PE matmul stream are the near-critical resources.  Pre-attention, the
V projection and k/q chunk-0 slices run ek-major, pipelined against
the serial input-DMA stream.  During attention, PE fill work rides the
exp-gated slack: c0 hosts the V tail + c1 k/q slices, c1 hosts the c2
slices, c2 hosts output-projection c0+c1 partials (staged to SBUF).
PSUM budget: scores 2x2 banks + 2 accumulator banks + one rotating
2-bank aux window.  The tail transposes c2's attention output on the
PE (identity matmul, keeps the p-state hot), re-injects the staged
partials via identity matmuls, and ships each [128, 673] E-chunk slab
as soon as it is ready."""
import numpy as np
import ml_dtypes
from contextlib import ExitStack
from itertools import chain

import concourse.bass as bass
import concourse.mybir as mybir
import concourse.tile as tile
from concourse import bacc
from concourse.bass import ts
from concourse.bass_utils import run_bass_kernel_spmd

B, T, E = 4, 1024, 768
H, D = 12, 64
HG = 6                  # heads per group
GD = HG * D             # 384 head-dims per group
BLOCK_M = 64
LIVE = 672              # tokens with (t % BLOCK_M) % 3 != 0
NQ = LIVE + 1           # live queries + one pinned-zero query (= mean-v row)
SCALE = 1.0 / 8.0       # 1/sqrt(64)
QC = 6                  # q chunks of 128 (last holds 33 live+pinned cols)
NPP = 5                 # rotating exp-output buffers

BF16 = mybir.dt.bfloat16
F32 = mybir.dt.float32

_perm = None
_nc = None

TRACE = False
LAST_RES = None


def _perm_live_first():
    t = np.arange(T)
    m = (t % BLOCK_M) % 3 == 0
    return np.concatenate([t[~m], t[m]])


def _build_bass(debug=False):
    nc = bacc.Bacc()
    xT_d = nc.dram_tensor("xT", [E, T], BF16, kind="ExternalInput")
    wq_d = nc.dram_tensor("wq", [E, GD], BF16, kind="ExternalInput")
    wk_d = nc.dram_tensor("wk", [E, GD], BF16, kind="ExternalInput")
    wv_d = nc.dram_tensor("wv", [E, GD], BF16, kind="ExternalInput")
    woT_d = nc.dram_tensor("woT", [GD, E], BF16, kind="ExternalInput")
    eye_d = nc.dram_tensor("eye", [128, 128], BF16, kind="ExternalInput")
    outT_d = nc.dram_tensor("outT", [E, NQ], BF16, kind="ExternalOutput")
    if debug:
        dbg = {nm: nc.dram_tensor(nm, sh, dt, kind="ExternalOutput")
               for nm, sh, dt in (
                   ("dbg_q", [128, 3, NQ], BF16),
                   ("dbg_k", [128, 3, T], BF16),
                   ("dbg_v", [128, 8, HG * (D + 1)], BF16),
                   ("dbg_attnT", [128, 3, QC * 128], BF16),
                   ("dbg_out01", [128, 3, NQ], BF16),
               )}

    xT_r = xT_d[:, :].rearrange("(c p) t -> p c t", p=128)
    wq_r = wq_d[:, :].rearrange("(c p) n -> p c n", p=128)
    wk_r = wk_d[:, :].rearrange("(c p) n -> p c n", p=128)
    wv_r = wv_d[:, :].rearrange("(c p) n -> p c n", p=128)
    woT_r = woT_d[:, :].rearrange("(c p) n -> p c n", p=128)

    Exp = mybir.ActivationFunctionType.Exp

    with tile.TileContext(nc) as tc:
        with tc.tile_pool(name="singles", bufs=1) as S:
            xT_sb = S.tile([128, 6, T], BF16)
            wq_sb = S.tile([128, 6, GD], BF16)
            wk_sb = S.tile([128, 6, GD], BF16)
            wv_sb = S.tile([128, 6, GD], BF16)
            woT_sb = S.tile([128, 3, E], BF16)
            eye_sb = S.tile([128, 128], BF16)
            warm_sb = S.tile([128, 512], BF16)
            qT_sb = S.tile([128, 3, NQ], BF16)
            kT_sb = S.tile([128, 3, T], BF16)
            v_sb = S.tile([128, 8, HG * (D + 1)], BF16)   # per-head [v | 1]
            attnT_sb = S.tile([128, 3, QC * 128], BF16)
            out01_sb = S.tile([128, 3, NQ], BF16)         # c0+c1 out partials
            dpre_sb = S.tile([1, 1], F32)
            ppb = [S.tile([128, 768], BF16, tag=f"ppb{j}", name=f"ppb{j}")
                   for j in range(NPP)]

            # ---- input DMAs (the DMA engines are one serial resource:
            # order by first consumer; k/v work starts as x chunks land,
            # wq gates the first exp, woT only the out-projection)
            nc.sync.dma_start(out=xT_sb[:, 0:2, :], in_=xT_r[:, 0:2, :])
            nc.sync.dma_start(out=wk_sb, in_=wk_r)
            nc.sync.dma_start(out=wv_sb, in_=wv_r)
            nc.sync.dma_start(out=xT_sb[:, 2:4, :], in_=xT_r[:, 2:4, :])
            nc.sync.dma_start(out=wq_sb, in_=wq_r)
            nc.sync.dma_start(out=xT_sb[:, 4:6, :], in_=xT_r[:, 4:6, :])
            nc.sync.dma_start(out=eye_sb, in_=eye_d[:, :])
            nc.sync.dma_start(out=woT_sb, in_=woT_r)

            nc.vector.memset(warm_sb, 0.5)
            for j in range(NPP):
                # pad cols so pv's last q-chunk stationary reads defined
                # small values (keeps garbage rows finite)
                nc.vector.memset(ppb[j][:, NQ:768], 1e-10)
            v_ones = v_sb[:, :, :].rearrange(
                "p a (h e) -> p a h e", e=D + 1)[:, :, :, D:D + 1]
            nc.vector.memset(v_ones, 1.0)
            # pinned-zero query column (mean-v row for masked tokens)
            nc.vector.memset(qT_sb[:, :, LIVE:NQ], 0.0)
            # preload the exp table while DMAs run
            nc.scalar.activation(dpre_sb, warm_sb[0:1, 0:1], Exp)

            # ---------- helpers ----------
            def vcopy(tt, t, eng):
                dst = v_sb[:, tt, :].rearrange(
                    "p (h e) -> p h e", e=D + 1)[:, :, 0:D]
                src = t[:, :].rearrange("p (h d) -> p h d", d=D)
                if eng == "vector":
                    nc.vector.tensor_copy(dst, src)
                else:
                    nc.scalar.copy(dst, src)

            def kq_copy(c, which, half, eng):
                n = T if which == "k" else LIVE
                s0, s1 = (0, 512) if half == 0 else (512, n)
                dstT = kT_sb if which == "k" else qT_sb
                return (nc.vector.tensor_copy if eng == "vector"
                        else nc.scalar.copy), dstT, s0, s1

            def kq_mms(p, c, which, half, ek):
                n = T if which == "k" else LIVE
                s0, s1 = (0, 512) if half == 0 else (512, n)
                w_sb = wk_sb if which == "k" else wq_sb
                nc.tensor.matmul(p[:, 0:s1 - s0],
                                 w_sb[:, ek, ts(c, 128)],
                                 xT_sb[:, ek, s0:s1],
                                 start=(ek == 0), stop=(ek == 5))

            def kq_half(pool, c, which, half, eng="vector"):
                """Generator: one half-slice of the k/q projection."""
                n = T if which == "k" else LIVE
                s0, s1 = (0, 512) if half == 0 else (512, n)
                dstT = kT_sb if which == "k" else qT_sb
                p = pool.tile([128, 512], F32, tag="aux",
                              name=f"kq_{which}{c}_{half}")
                for ek in range(6):
                    kq_mms(p, c, which, half, ek)
                    if ek == 2:
                        yield
                if eng == "vector":
                    nc.vector.tensor_copy(dstT[:, c, s0:s1], p[:, 0:s1 - s0])
                else:
                    nc.scalar.copy(dstT[:, c, s0:s1], p[:, 0:s1 - s0])
                yield

            def v_tt(pool, tt, eng="vector"):
                t = pool.tile([128, GD], F32, tag="aux", name=f"vt{tt}")
                for ek in range(6):
                    nc.tensor.matmul(t, xT_sb[:, ek, ts(tt, 128)],
                                     wv_sb[:, ek, :],
                                     start=(ek == 0), stop=(ek == 5))
                    if ek == 2:
                        yield
                vcopy(tt, t, eng)
                yield

            def op_prerun(pool, ec):
                """c0+c1 contribution of out-projection E-chunk ec, staged
                to SBUF bf16 via two 1-bank tiles (c2 joins in the tail)."""
                for s0, s1, nm in ((0, 512, "A"), (512, NQ, "B")):
                    p = pool.tile([128, 512], F32, tag="aux",
                                  name=f"opp{ec}{nm}")
                    for c3 in (0, 1):
                        nc.tensor.matmul(p[:, 0:s1 - s0],
                                         woT_sb[:, c3, ts(ec, 128)],
                                         attnT_sb[:, c3, s0:s1],
                                         start=(c3 == 0), stop=(c3 == 1),
                                         skip_group_check=True)
                    nc.vector.tensor_copy(out01_sb[:, ec, s0:s1],
                                          p[:, 0:s1 - s0])
                    yield

            # ---------- pre-attention: warm, K c0 first half, V tts 0-5,
            # Q c0 — all ek-major, pipelined against the x-chunk DMAs ----
            with ExitStack() as pre:
                kqp = pre.enter_context(
                    tc.tile_pool(name="kqp", bufs=2, space="PSUM"))
                vap = pre.enter_context(
                    tc.tile_pool(name="vap", bufs=1, space="PSUM"))
                warm_t = kqp.tile([128, 512], F32, tag="aux", name="warm_t")
                for _ in range(5):
                    nc.tensor.matmul(warm_t[:, 0:384], warm_sb[:, 0:128],
                                     warm_sb[:, 0:384], start=True, stop=True)
                ka = kqp.tile([128, 512], F32, tag="aux", name="ka")
                vt = [vap.tile([128, GD], F32, tag=f"v{tt}", name=f"vt{tt}")
                      for tt in range(6)]
                for ek in range(6):
                    kq_mms(ka, 0, "k", 0, ek)
                    for tt in range(6):
                        nc.tensor.matmul(vt[tt], xT_sb[:, ek, ts(tt, 128)],
                                         wv_sb[:, ek, :],
                                         start=(ek == 0), stop=(ek == 5))
                nc.vector.tensor_copy(kT_sb[:, 0, 0:512], ka)
                qa = kqp.tile([128, 512], F32, tag="aux", name="qa")
                for ek in range(6):
                    kq_mms(qa, 0, "q", 0, ek)
                nc.scalar.copy(qT_sb[:, 0, 0:512], qa)
                qb = kqp.tile([128, 512], F32, tag="aux", name="qb")
                for ek in range(6):
                    kq_mms(qb, 0, "q", 1, ek)
                nc.scalar.copy(qT_sb[:, 0, 512:LIVE], qb[:, 0:LIVE - 512])
                # v copies alternate DVE/ACT; tt0/tt1 first (their banks are
                # the first ones the attention pools reuse)
                for tt in range(6):
                    vcopy(tt, vt[tt], "vector" if tt % 2 == 0 else "scalar")

            with ExitStack() as stack:
                ps_s = stack.enter_context(
                    tc.tile_pool(name="ps_s", bufs=2, space="PSUM"))
                ps_acc = stack.enter_context(
                    tc.tile_pool(name="ps_acc", bufs=1, space="PSUM"))
                att_pool = stack.enter_context(
                    tc.tile_pool(name="att", bufs=2))
                rq_pool = stack.enter_context(
                    tc.tile_pool(name="rq", bufs=2))
                ppi = [0]
                atts = {}

                def attention_c(c, fill, fill_quota, transpose=True):
                    accs = [ps_acc.tile([128, QC * (D + 1)], F32,
                                        tag=f"acc{i}", name=f"acc{c}_{i}")
                            for i in range(2)]

                    def pv(kt, i, pp):
                        h = 2 * c + i
                        vh = v_sb[:, kt, h * (D + 1):(h + 1) * (D + 1)]
                        for qc in range(QC):
                            nc.tensor.matmul(
                                accs[i][:, qc * (D + 1):(qc + 1) * (D + 1)],
                                pp[:, qc * 128:(qc + 1) * 128],
                                vh,
                                # one start=True per psum bank: it clears the
                                # whole bank's has_written so later regions'
                                # first writes overwrite rather than add
                                start=(kt == 0 and qc == 0), stop=(kt == 7),
                                skip_group_check=True)

                    pend = []
                    for kt in range(8):
                        for i in range(2):      # head 2c+i
                            po = i * 64
                            kh = kT_sb[po:po + 64, c, ts(kt, 128)]
                            qh = qT_sb[po:po + 64, c, :]
                            sp = ps_s.tile([128, T], F32, tag="s",
                                           name=f"s{c}_{kt}_{i}")
                            nc.tensor.matmul(sp[:, 0:512], kh, qh[:, 0:512],
                                             start=True, stop=True)
                            nc.tensor.matmul(sp[:, 512:NQ], kh, qh[:, 512:NQ],
                                             start=True, stop=True)
                            pp = ppb[ppi[0] % NPP]
                            ppi[0] += 1
                            nc.scalar.activation(pp[:, 0:NQ], sp[:, 0:NQ],
                                                 Exp, scale=SCALE)
                            pend.append((kt, i, pp))
                            while len(pend) > 3:
                                pv(*pend.pop(0))
                            for _ in range(fill_quota):
                                try:
                                    next(fill)
                                except StopIteration:
                                    break
                    # ---- normalize: per-partition recip + strided multiply
                    att = att_pool.tile([128, QC * 128], BF16, tag="att")
                    atts[c] = att

                    def norm(i):
                        rq = rq_pool.tile([128, QC], F32, tag=f"rq{i}")
                        a = accs[i]
                        den = bass.AP(tensor=a.tensor, offset=a.offset + D,
                                      ap=[list(a.ap[0])] + [[D + 1, QC]])
                        nc.vector.reciprocal(rq, den)
                        src = bass.AP(tensor=a.tensor, offset=a.offset,
                                      ap=[list(a.ap[0])] + [[D + 1, QC],
                                                            [1, D]])
                        sca = bass.AP(tensor=rq.tensor, offset=rq.offset,
                                      ap=[list(rq.ap[0])] + [[1, QC],
                                                             [0, D]])
                        dst = bass.AP(tensor=att.tensor,
                                      offset=att.offset + i * 64,
                                      ap=[list(att.ap[0])] + [[128, QC],
                                                              [1, D]])
                        nc.vector.tensor_mul(dst, src, sca)

                    while pend:
                        kt_, i_, pp_ = pend.pop(0)
                        pv(kt_, i_, pp_)
                        if not any(e[1] == i_ for e in pend):
                            norm(i_)
                    if transpose:
                        # -> attnT [dims, tok] on the XBAR DMA path (idle
                        # mid-kernel)
                        o = attnT_sb[:, c, :]
                        o3 = bass.AP(tensor=o.tensor, offset=o.offset,
                                     ap=[list(o.ap[0])] + [[128, QC],
                                                           [1, 128]])
                        nc.sync.dma_start_transpose(o3, att[:, :])

                # c0 fills: k chunk0 2nd half, V tts 6-7, then c1 k/q
                # slices (1-bank tiles rotating through the 2-slot window)
                with ExitStack() as auxst:
                    vtp = auxst.enter_context(
                        tc.tile_pool(name="vtp", bufs=2, space="PSUM"))
                    f0 = chain(kq_half(vtp, 0, "k", 1),
                               v_tt(vtp, 6, "vector"),
                               v_tt(vtp, 7, "scalar"),
                               kq_half(vtp, 1, "k", 0),
                               kq_half(vtp, 1, "k", 1),
                               kq_half(vtp, 1, "q", 0, "scalar"),
                               kq_half(vtp, 1, "q", 1, "scalar"))
                    attention_c(0, f0, 2)
                    for _ in f0:
                        pass
                with ExitStack() as auxst:
                    kq2 = auxst.enter_context(
                        tc.tile_pool(name="kq2", bufs=2, space="PSUM"))
                    f1 = chain(kq_half(kq2, 2, "k", 0),
                               kq_half(kq2, 2, "k", 1),
                               kq_half(kq2, 2, "q", 0, "scalar"),
                               kq_half(kq2, 2, "q", 1, "scalar"))
                    attention_c(1, f1, 2)
                    for _ in f1:
                        pass
                with ExitStack() as auxst:
                    opp = auxst.enter_context(
                        tc.tile_pool(name="opp", bufs=2, space="PSUM"))
                    f2 = chain(*[op_prerun(opp, ec) for ec in range(3)])
                    attention_c(2, f2, 2, transpose=False)
                    for _ in f2:
                        pass
                # ---- c2 transpose on the PE (identity matmul): keeps the
                # p-state hot through the norm latency; psum copies out on
                # DVE + ACT halves
                with tc.tile_pool(name="tp", bufs=1, space="PSUM") as tp:
                    tp_t = tp.tile([128, QC * 128], BF16, tag="t",
                                   name="tp_t")
                    att2 = atts[2]
                    for j in range(QC):
                        nc.tensor.matmul(tp_t[:, j * 128:(j + 1) * 128],
                                         att2[:, j * 128:(j + 1) * 128],
                                         eye_sb,
                                         is_transpose=True,
                                         start=(j == 0), stop=True,
                                         skip_group_check=True)
                    nc.vector.tensor_copy(attnT_sb[:, 2, 0:384],
                                          tp_t[:, 0:384])
                    nc.scalar.copy(attnT_sb[:, 2, 384:768],
                                   tp_t[:, 384:768])

            if debug:
                for nm, t in (("dbg_q", qT_sb), ("dbg_k", kT_sb),
                              ("dbg_v", v_sb), ("dbg_attnT", attnT_sb),
                              ("dbg_out01", out01_sb)):
                    nc.sync.dma_start(out=dbg[nm][:, :, :], in_=t[:, :, :])

            # ---------- tail: finish out-projection per E-chunk ----------
            with tc.tile_pool(name="opf", bufs=3, space="PSUM") as opf, \
                 tc.tile_pool(name="obp", bufs=3) as obp:
                ps = []
                for ec in range(3):
                    # re-inject the staged c0+c1 partial via identity matmul
                    # (runs before attnT c2 is ready -> PE stays busy)
                    p = opf.tile([128, NQ], F32, tag="o", name=f"of{ec}")
                    for s0, s1 in ((0, 512), (512, NQ)):
                        nc.tensor.matmul(p[:, s0:s1], eye_sb,
                                         out01_sb[:, ec, s0:s1],
                                         start=True, stop=False,
                                         skip_group_check=True)
                    ps.append(p)
                for ec in range(6):
                    if ec < 3:
                        p = ps[ec]
                        cs = (2,)
                    else:
                        p = opf.tile([128, NQ], F32, tag="o", name=f"of{ec}")
                        cs = (0, 1, 2)
                    for c3 in cs:
                        for s0, s1 in ((0, 512), (512, NQ)):
                            nc.tensor.matmul(p[:, s0:s1],
                                             woT_sb[:, c3, ts(ec, 128)],
                                             attnT_sb[:, c3, s0:s1],
                                             start=(ec >= 3 and c3 == 0),
                                             stop=(c3 == 2),
                                             skip_group_check=True)
                    ob = obp.tile([128, NQ], BF16, tag="ob", name=f"ob{ec}")
                    if ec % 2 == 0:
                        nc.vector.tensor_copy(ob, p)
                    else:
                        nc.scalar.copy(ob, p)
                    nc.sync.dma_start(out=outT_d[ts(ec, 128), :], in_=ob)

    nc.finalize()
    return nc


def _get_bass():
    global _nc
    if _nc is None:
        _nc = _build_bass()
    return _nc


def kernel(x, idx, struct_embed, w_qkv, w_out, b_out):
    global _perm
    if _perm is None:
        _perm = _perm_live_first()
    perm = _perm

    x = np.asarray(x, dtype=np.float32)
    idx = np.asarray(idx)
    struct_embed = np.asarray(struct_embed, dtype=np.float32)
    w_qkv = np.asarray(w_qkv, dtype=np.float32)
    w_out = np.asarray(w_out, dtype=np.float32)
    b_out = np.asarray(b_out, dtype=np.float32)

    sid = ((idx == 1) * 1 + (idx == 2) * 2 + (idx == 3) * 3)  # [B,T]
    xs = x + struct_embed[sid]                                # fp32 pre-add

    bf = ml_dtypes.bfloat16
    eye = np.eye(128, dtype=bf)
    in_maps = []
    for core in range(8):
        b, g = core // 2, core % 2
        sl = slice(g * GD, (g + 1) * GD)
        in_maps.append({
            "xT": np.ascontiguousarray(xs[b].T[:, perm]).astype(bf),
            "wq": np.ascontiguousarray(w_qkv[0 * E:1 * E][sl].T).astype(bf),
            "wk": np.ascontiguousarray(w_qkv[1 * E:2 * E][sl].T).astype(bf),
            "wv": np.ascontiguousarray(w_qkv[2 * E:3 * E][sl].T).astype(bf),
            "woT": np.ascontiguousarray(w_out[:, sl].T).astype(bf),
            "eye": eye,
        })

    res = run_bass_kernel_spmd(_get_bass(), in_maps, core_ids=list(range(8)),
                               trace=TRACE)
    if TRACE:
        global LAST_RES
        LAST_RES = res

    inv = np.empty(T, dtype=np.int64)
    inv[perm] = np.arange(T)
    out = np.empty((B, T, E), dtype=np.float32)
    full = np.empty((T, E), dtype=np.float32)
    for b in range(B):
        acc = (res.results[2 * b]["outT"].astype(np.float32)
               + res.results[2 * b + 1]["outT"].astype(np.float32))
        # device computes permuted cols 0:NQ of outT; all masked-token
        # rows equal col LIVE (the pinned-zero query = mean-v output)
        full[0:NQ] = acc.T
        full[NQ:] = acc[:, LIVE]
        out[b] = full[inv] + b_out[None, :]
    return out
